# revision 1
# baseline (speedup 1.0000x reference)
"""ArchetypalNeuralMemory on 8 TRN2 NeuronCores (Bass/Tile).

Strategy (sharding_hint: data-parallel over B, replicate fast weights):
  - token sharding: core c owns batch b=c//2, parity p=c%2 -> tokens
    x[b, p::2, :]  (2048 tokens; 32 rows of every one of the 64 chunks).
  - projections (k,v,q,gates) computed on the local shard, k/kT/v and
    chunk-mean partials all-gathered (device collective).
  - the 64-step fast-weight scan is inherently serial (each chunk's
    gradient feeds the next chunk's weights) and per-step collectives
    cost >= 10us, so the scan is REPLICATED on every core; only the
    retrieval (P1/P2) + output projection are sharded over tokens.
  - matmul operands bf16 (full PE rate); W0/W1 masters fp32 in SBUF,
    updated with fused scalar_tensor_tensor; momentum in bf16 via a
    PE diag(eta) matmul accumulated straight into the gradient PSUM.

kernel(**inputs) takes FULL unsharded inputs, returns FULL output.
"""

import sys

if "/opt/trn_rl_repo" not in sys.path:
    sys.path.insert(0, "/opt/trn_rl_repo")

import numpy as np

B, S, D = 4, 4096, 512
C = 64            # chunk length
NSTEP = 64        # chunks
NCORE = 8
TOK = 2048        # tokens per core
NT = TOK // 128   # 16 row tiles per core
LR, MOM, DEC = 0.1, 0.9, 0.01
EPS_RMS = 1.1920929e-07
INV_N = 2.0 / (B * C * D)

_BUILT = {}


def _build(n_steps=NSTEP):
    import concourse.bacc as bacc
    import concourse.mybir as mybir
    import concourse.tile as tile
    from contextlib import ExitStack

    F32 = mybir.dt.float32
    BF16 = mybir.dt.bfloat16
    AF = mybir.ActivationFunctionType
    ALU = mybir.AluOpType
    AX = mybir.AxisListType

    nc = bacc.Bacc("TRN2", target_bir_lowering=False)
    P = nc.declare_dram_parameter

    xs_d = P("xs", [TOK, D], F32, isOutput=False)
    mb_d = P("Mb", [D, D], BF16, isOutput=False)        # gs-scaled M[b], [d,e]
    wkT_d = P("WkT", [D, D], BF16, isOutput=False)       # [e, e']
    wvT_d = P("WvT", [D, D], BF16, isOutput=False)       # gs-folded, [d, e]
    wqT_d = P("WqT", [D, D], BF16, isOutput=False)       # gr-folded, [d, e]
    woT_d = P("WoutT", [D, D], BF16, isOutput=False)     # [d, e]
    wgT_d = [P(n, [D, D], BF16, isOutput=False) for n in ("WgdT", "WglT", "WgmT")]
    bg_d = [P(n, [D, 1], F32, isOutput=False) for n in ("bgd", "bgl", "bgm")]
    w0t32_d = P("W0T32", [D, D], F32, isOutput=False)    # mem_W[0].T
    w0t16_d = P("W0T16", [D, D], BF16, isOutput=False)
    w1n32_d = P("W1n32", [D, D], F32, isOutput=False)    # mem_W[1]
    w1n16_d = P("W1n16", [D, D], BF16, isOutput=False)
    w1t16_d = P("W1T16", [D, D], BF16, isOutput=False)   # mem_W[1].T
    eye16_d = P("EYE16", [128, 128], BF16, isOutput=False)
    eye32_d = P("EYE32", [128, 128], F32, isOutput=False)
    out_d = P("out", [TOK, D], F32, isOutput=True)

    with tile.TileContext(nc) as tc, ExitStack() as ctx:
        dram = ctx.enter_context(tc.tile_pool(name="dram", bufs=1, space="DRAM"))
        ps2 = ctx.enter_context(tc.tile_pool(name="ps2", bufs=2, space="PSUM"))
        ps4 = ctx.enter_context(tc.tile_pool(name="ps4", bufs=4, space="PSUM"))
        pers = ctx.enter_context(tc.tile_pool(name="pers", bufs=1))
        st8 = ctx.enter_context(tc.tile_pool(name="st8", bufs=6))

        def dmaload(pool, shape, dt, src, tag):
            t = pool.tile(shape, dt, tag=tag, name=tag)
            nc.sync.dma_start(t[:], src)
            return t

        # ---------- persistent state ----------
        eye16 = dmaload(pers, [128, 128], BF16, eye16_d[:], "eye16")
        w0t32 = [dmaload(pers, [128, D], F32, w0t32_d[128 * i : 128 * (i + 1), :], f"w0t32_{i}") for i in range(4)]
        w1n32 = [dmaload(pers, [128, D], F32, w1n32_d[128 * i : 128 * (i + 1), :], f"w1n32_{i}") for i in range(4)]
        w0t16 = [dmaload(pers, [128, D], BF16, w0t16_d[128 * i : 128 * (i + 1), :], f"w0t16_{i}") for i in range(4)]
        w1n16 = [dmaload(pers, [128, D], BF16, w1n16_d[128 * i : 128 * (i + 1), :], f"w1n16_{i}") for i in range(4)]
        w1t16 = [dmaload(pers, [128, D], BF16, w1t16_d[128 * i : 128 * (i + 1), :], f"w1t16_{i}") for i in range(4)]
        avc = pers.tile([128, NSTEP], F32, tag="avc", name="avc")
        evc = pers.tile([128, NSTEP], F32, tag="evc", name="evc")
        cvc = pers.tile([128, NSTEP], F32, tag="cvc", name="cvc")
        eps_t = pers.tile([128, 1], F32, tag="eps", name="eps")
        nc.vector.memset(eps_t[:], EPS_RMS)
        m0 = [st8.tile([128, D], BF16, tag="m0n", name="m0n") for _ in range(4)]
        m1 = [st8.tile([128, D], BF16, tag="m1n", name="m1n") for _ in range(4)]
        for i in range(4):
            nc.vector.memset(m0[i][:], 0.0)
            nc.vector.memset(m1[i][:], 0.0)

        # dram buffers
        kT_sh = dram.tile([D, TOK], BF16)
        k_sh = dram.tile([TOK, D], BF16)
        v_sh = dram.tile([TOK, D], BF16)
        cm_sh = dram.tile([D, NSTEP], F32)
        qT_d = dram.tile([D, TOK], BF16)
        ret_d = dram.tile([TOK, D], BF16)
        kTG = dram.tile([D * NCORE, TOK], BF16, addr_space="Shared")
        kG = dram.tile([TOK * NCORE, D], BF16, addr_space="Shared")
        vG = dram.tile([TOK * NCORE, D], BF16, addr_space="Shared")
        cmG = dram.tile([D * NCORE, NSTEP], F32, addr_space="Shared")
        gbounce = dram.tile([1, 3 * NSTEP], F32)

        def tr128(dst, src_tile, cols, eye, n=4):
            """transpose n [128-col] blocks of src into dst slices (via psum)."""
            for i in range(n):
                tp = ps2.tile([128, 128], src_tile.dtype, tag="ptr", name="ptr")
                nc.tensor.transpose(tp[:], src_tile[:, 128 * i : 128 * (i + 1)], eye)
                nc.vector.tensor_copy(dst[:, 128 * i : 128 * (i + 1)], tp[:])

        def sh3(dram_t, rt):
            """[p, i, j] view of a [512, 2048] dram tensor at col block rt."""
            return dram_t[:].rearrange("(i p) c -> p i c", p=128)[:, :, 128 * rt : 128 * (rt + 1)]

        # ---------- phase A ----------
        with tc.tile_pool(name="pa", bufs=1) as pa, tc.tile_pool(name="par", bufs=2) as par:
            eye32 = dmaload(pa, [128, 128], F32, eye32_d[:], "eye32")
            mb = [dmaload(pa, [128, D], BF16, mb_d[128 * i : 128 * (i + 1), :], f"mb{i}") for i in range(4)]
            wkT = [dmaload(pa, [128, D], BF16, wkT_d[128 * i : 128 * (i + 1), :], f"wkT{i}") for i in range(4)]
            wvT = [dmaload(pa, [128, D], BF16, wvT_d[128 * i : 128 * (i + 1), :], f"wvT{i}") for i in range(4)]
            wqT = [dmaload(pa, [128, D], BF16, wqT_d[128 * i : 128 * (i + 1), :], f"wqT{i}") for i in range(4)]
            wg = [[dmaload(pa, [128, D], BF16, wgT_d[g][128 * i : 128 * (i + 1), :], f"wg{g}_{i}")
                   for i in range(4)] for g in range(3)]
            bg = [[dmaload(pa, [128, 1], F32, bg_d[g][128 * i : 128 * (i + 1), :], f"bg{g}_{i}")
                   for i in range(4)] for g in range(3)]
            cmp_ = [pa.tile([128, NSTEP], F32, tag=f"cmp{i}", name=f"cmp{i}") for i in range(4)]

            def mm4(lhsT_tile, rhs_tiles, out_shape=None):
                pm = ps2.tile([128, D], F32, tag="pmm", name="pmm")
                for kk in range(4):
                    nc.tensor.matmul(pm[:], lhsT_tile[:, 128 * kk : 128 * (kk + 1)], rhs_tiles[kk][:],
                                     start=(kk == 0), stop=(kk == 3))
                return pm

            def l2tile(pm, silu_scale):
                kp = par.tile([128, D], BF16, tag="kp", name="kp")
                if silu_scale is None:
                    nc.scalar.activation(kp[:], pm[:], AF.Silu)
                else:
                    nc.scalar.activation(kp[:], pm[:], AF.Silu, scale=silu_scale)
                scr = par.tile([128, D], F32, tag="l2scr", name="l2scr")
                ss2 = par.tile([128, 1], F32, tag="ss2", name="ss2")
                nc.vector.tensor_tensor(scr[:], kp[:], kp[:], ALU.mult)
                nc.vector.tensor_reduce(ss2[:], scr[:], AX.X, ALU.add)
                nr2 = par.tile([128, 1], F32, tag="nr2", name="nr2")
                nc.scalar.activation(nr2[:], ss2[:], AF.Sqrt)
                rs = par.tile([128, 1], F32, tag="rs", name="rs")
                nc.vector.reciprocal(rs[:], nr2[:])
                kn = par.tile([128, D], BF16, tag="kn", name="kn")
                nc.vector.tensor_scalar(kn[:], kp[:], rs[:], None, ALU.mult)
                return kn

            for rt in range(NT):
                xt = dmaload(par, [128, D], F32, xs_d[128 * rt : 128 * (rt + 1), :], "xt")
                scr0 = par.tile([128, D], F32, tag="scr0", name="scr0")
                ss = par.tile([128, 1], F32, tag="ss", name="ss")
                nc.vector.tensor_tensor(scr0[:], xt[:], xt[:], ALU.mult)
                nc.vector.tensor_reduce(ss[:], scr0[:], AX.X, ALU.add)
                nrm = par.tile([128, 1], F32, tag="nrm", name="nrm")
                nc.scalar.activation(nrm[:], ss[:], AF.Sqrt, scale=1.0 / D, bias=eps_t[:])
                rinv = par.tile([128, 1], F32, tag="rinv", name="rinv")
                nc.vector.reciprocal(rinv[:], nrm[:])

                xtT = par.tile([128, D], BF16, tag="xtT", name="xtT")
                tr128(xtT, xt, None, eye32[:])
                # chunk sums for gates (raw x, bf16): chunks 4rt..4rt+3
                for dt in range(4):
                    nc.vector.tensor_reduce(
                        cmp_[dt][:, 4 * rt : 4 * rt + 4],
                        xtT[:, 128 * dt : 128 * (dt + 1)].rearrange("p (t j) -> p t j", j=32),
                        AX.X, ALU.add)

                # t1 = silu(rinv * (x @ Mb)); transposed small
                pm = mm4(xtT, mb)
                t1s = par.tile([128, D], BF16, tag="t1s", name="t1s")
                nc.scalar.activation(t1s[:], pm[:], AF.Silu, scale=rinv[:])
                t1sT = par.tile([128, D], BF16, tag="t1sT", name="t1sT")
                tr128(t1sT, t1s, None, eye16[:])

                # k
                pmk = mm4(t1sT, wkT)
                kn = l2tile(pmk, None)
                nc.sync.dma_start(k_sh[128 * rt : 128 * (rt + 1), :], kn[:])
                knT = par.tile([128, D], BF16, tag="knT", name="knT")
                tr128(knT, kn, None, eye16[:])
                nc.sync.dma_start(sh3(kT_sh, rt), knT[:].rearrange("p (i j) -> p i j", j=128))

                # v
                pmv = mm4(xtT, wvT)
                vt_ = par.tile([128, D], BF16, tag="vtile", name="vtile")
                nc.scalar.activation(vt_[:], pmv[:], AF.Silu, scale=rinv[:])
                nc.sync.dma_start(v_sh[128 * rt : 128 * (rt + 1), :], vt_[:])

                # q
                pmq = mm4(xtT, wqT)
                qn = l2tile(pmq, rinv[:])
                qnT = par.tile([128, D], BF16, tag="qnT", name="qnT")
                tr128(qnT, qn, None, eye16[:])
                nc.sync.dma_start(sh3(qT_d, rt), qnT[:].rearrange("p (i j) -> p i j", j=128))

            for dt in range(4):
                nc.sync.dma_start(cm_sh[128 * dt : 128 * (dt + 1), :], cmp_[dt][:])

            # ---------- all-gathers ----------
            import os as _os
            if _os.environ.get("KERNEL_SKIP_AG") == "1":
                for src, dst in ((kT_sh, kTG), (k_sh, kG), (v_sh, vG), (cm_sh, cmG)):
                    nc.gpsimd.dma_start(dst[0 : src.shape[0], :], src[:])
            else:
                ncoll = int(_os.environ.get("KERNEL_NCOLL", "4"))
                pairs = ((kT_sh, kTG), (k_sh, kG), (v_sh, vG), (cm_sh, cmG))
                for i, (src, dst) in enumerate(pairs):
                    if i < ncoll:
                        nc.gpsimd.collective_compute(
                            "AllGather", ALU.bypass, replica_groups=[list(range(NCORE))],
                            ins=[src.opt()], outs=[dst.opt()])
                    else:
                        nc.gpsimd.dma_start(dst[0 : src.shape[0], :], src[:])

            # ---------- gates (replicated) ----------
            cmT = [pa.tile([128, B * NSTEP], BF16, tag=f"cmT{i}", name=f"cmT{i}") for i in range(4)]
            for dt in range(4):
                for b in range(B):
                    tmp0 = par.tile([128, NSTEP], F32, tag="cmg0", name="cmg0")
                    tmp1 = par.tile([128, NSTEP], F32, tag="cmg1", name="cmg1")
                    nc.sync.dma_start(tmp0[:], cmG[D * (2 * b) + 128 * dt : D * (2 * b) + 128 * (dt + 1), :])
                    nc.sync.dma_start(tmp1[:], cmG[D * (2 * b + 1) + 128 * dt : D * (2 * b + 1) + 128 * (dt + 1), :])
                    nc.vector.tensor_tensor(
                        cmT[dt][:, NSTEP * b : NSTEP * (b + 1)], tmp0[:], tmp1[:], ALU.add)

            ones16 = pa.tile([128, 1], BF16, tag="ones", name="ones")
            nc.vector.memset(ones16[:], 1.0)
            gvec = []
            for g in range(3):
                gT = []
                for et in range(4):
                    pm = ps2.tile([128, B * NSTEP], F32, tag="pmm", name="pmm")
                    for dt in range(4):
                        nc.tensor.matmul(
                            pm[:], wg[g][dt][:, 128 * et : 128 * (et + 1)], cmT[dt][:],
                            start=(dt == 0), stop=(dt == 3))
                    gt = par.tile([128, B * NSTEP], BF16, tag=f"gT{et}", name=f"gT{et}")
                    nc.scalar.activation(gt[:], pm[:], AF.Sigmoid, bias=bg[g][et][:])
                    gT.append(gt)
                ps_s = ps2.tile([1, B * NSTEP], F32, tag="pmm", name="pmm")
                for et in range(4):
                    nc.tensor.matmul(ps_s[:], ones16[:], gT[et][:], start=(et == 0), stop=(et == 3))
                svf = par.tile([1, B * NSTEP], F32, tag=f"svf{g}", name=f"svf{g}")
                nc.vector.tensor_copy(svf[:], ps_s[:])
                sv = par.tile([1, NSTEP], F32, tag=f"sv{g}", name=f"sv{g}")
                t01 = par.tile([1, NSTEP], F32, tag="t01", name="t01")
                nc.vector.tensor_tensor(t01[:], svf[:, 0:NSTEP], svf[:, NSTEP : 2 * NSTEP], ALU.add)
                nc.vector.tensor_tensor(sv[:], svf[:, 2 * NSTEP : 3 * NSTEP], svf[:, 3 * NSTEP :], ALU.add)
                nc.vector.tensor_tensor(sv[:], t01[:], sv[:], ALU.add)
                gvec.append(sv)

            SM = 1.0 / (D * B)
            fin = par.tile([1, 3 * NSTEP], F32, tag="fin", name="fin")
            nc.vector.tensor_scalar(fin[:, 0:NSTEP], gvec[0][:], -DEC * SM, 1.0, ALU.mult, ALU.add)
            nc.vector.tensor_scalar(fin[:, NSTEP : 2 * NSTEP], gvec[2][:], MOM * SM, None, ALU.mult)
            nc.vector.tensor_scalar(fin[:, 2 * NSTEP :], gvec[1][:], -LR * INV_N * SM, None, ALU.mult)
            nc.sync.dma_start(gbounce[:], fin[:])
            finb = par.tile([1, 3 * NSTEP], F32, tag="finb", name="finb")
            nc.sync.dma_start(finb[:], gbounce[:])
            nc.gpsimd.partition_broadcast(avc[:], finb[:, 0:NSTEP])
            nc.gpsimd.partition_broadcast(evc[:], finb[:, NSTEP : 2 * NSTEP])
            nc.gpsimd.partition_broadcast(cvc[:], finb[:, 2 * NSTEP :])

        # ---------- phase B: the scan ----------
        with tc.tile_pool(name="rot", bufs=2) as rot, tc.tile_pool(name="rot3", bufs=3) as rot3:
            for t in range(n_steps):
                tsl = slice(32 * t, 32 * (t + 1))

                ktT = rot3.tile([128, 1024], BF16, tag="ktT", name="ktT")
                ktT_v = ktT[:].rearrange("p (dt g) -> p dt g", g=256)
                for c in range(NCORE):
                    src = kTG[D * c : D * (c + 1), tsl].rearrange("(dt p) j -> p dt j", p=128)
                    nc.sync.dma_start(ktT_v[:, :, 32 * c : 32 * (c + 1)], src)
                kt = [rot3.tile([128, D], BF16, tag=f"kt{rh}", name=f"kt{rh}") for rh in range(2)]
                vt = [rot3.tile([128, D], BF16, tag=f"vt{rh}", name=f"vt{rh}") for rh in range(2)]
                for c in range(NCORE):
                    rh, ro = c // 4, 32 * (c % 4)
                    nc.sync.dma_start(kt[rh][ro : ro + 32, :], kG[TOK * c + 32 * t : TOK * c + 32 * t + 32, :])
                    nc.sync.dma_start(vt[rh][ro : ro + 32, :], vG[TOK * c + 32 * t : TOK * c + 32 * t + 32, :])
                qtT = rot3.tile([128, 128], BF16, tag="qtT", name="qtT")
                nc.sync.dma_start(
                    qtT[:].rearrange("p (dt j) -> p dt j", j=32),
                    qT_d[:, tsl].rearrange("(dt p) j -> p dt j", p=128))

                eyeet = rot.tile([128, 128], BF16, tag="eyeet", name="eyeet")
                nc.vector.tensor_scalar(eyeet[:], eye16[:], evc[:, t : t + 1], None, ALU.mult)

                # P1/P2: retrieval for the local 32 rows (pre-update weights)
                p_hq = ps2.tile([32, D], F32, tag="pmm", name="pmm")
                for dt in range(4):
                    nc.tensor.matmul(p_hq[:], qtT[:, 32 * dt : 32 * (dt + 1)], w0t16[dt][:],
                                     start=(dt == 0), stop=(dt == 3))
                sq = rot.tile([32, D], BF16, tag="sq", name="sq")
                nc.scalar.activation(sq[:], p_hq[:], AF.Silu)
                p_sqT = ps2.tile([128, 128], BF16, tag="ptr", name="ptr")
                for it in range(4):
                    nc.tensor.transpose(p_sqT[:, 32 * it : 32 * (it + 1)],
                                        sq[:, 128 * it : 128 * (it + 1)], eye16[0:32, 0:32])
                sqT = rot.tile([128, 128], BF16, tag="sqT", name="sqT")
                nc.vector.tensor_copy(sqT[:], p_sqT[:])
                p_ret = ps2.tile([32, D], F32, tag="pmm", name="pmm")
                for it in range(4):
                    nc.tensor.matmul(p_ret[:], sqT[:, 32 * it : 32 * (it + 1)], w1t16[it][:],
                                     start=(it == 0), stop=(it == 3))
                rsb = rot.tile([32, D], BF16, tag="rsb", name="rsb")
                nc.scalar.activation(rsb[:], p_ret[:], AF.Copy)
                nc.sync.dma_start(ret_d[tsl, :], rsb[:])

                # P3: h1 = k_t @ W0^T  (natural, [256, 512])
                p_h1 = [ps2.tile([128, D], F32, tag="pmm", name="pmm") for _ in range(2)]
                for rh in range(2):
                    for dt in range(4):
                        nc.tensor.matmul(p_h1[rh][:], ktT[:, 256 * dt + 128 * rh : 256 * dt + 128 * (rh + 1)],
                                         w0t16[dt][:], start=(dt == 0), stop=(dt == 3))
                a1 = [rot.tile([128, D], BF16, tag=f"a1_{rh}", name=f"a1_{rh}") for rh in range(2)]
                ds = [rot.tile([128, D], BF16, tag=f"ds_{rh}", name=f"ds_{rh}") for rh in range(2)]
                for rh in range(2):
                    nc.scalar.activation(a1[rh][:], p_h1[rh][:], AF.Silu)
                    nc.scalar.activation(ds[rh][:], p_h1[rh][:], AF.Derivative_silu)
                a1c = [rot.tile([128, D], BF16, tag=f"a1c_{rh}", name=f"a1c_{rh}") for rh in range(2)]
                for rh in range(2):
                    nc.vector.tensor_scalar(a1c[rh][:], a1[rh][:], cvc[:, t : t + 1], None, ALU.mult)

                a1T = rot.tile([128, 1024], BF16, tag="a1T", name="a1T")
                for it in range(4):
                    tp = ps2.tile([128, 256], BF16, tag="ptr", name="ptr")
                    for rh in range(2):
                        nc.tensor.transpose(tp[:, 128 * rh : 128 * (rh + 1)],
                                            a1[rh][:, 128 * it : 128 * (it + 1)], eye16[:])
                    nc.vector.tensor_copy(a1T[:, 256 * it : 256 * (it + 1)], tp[:])

                # P4: y
                p_y = [ps2.tile([128, D], F32, tag="pmm", name="pmm") for _ in range(2)]
                for rh in range(2):
                    for it in range(4):
                        nc.tensor.matmul(p_y[rh][:], a1T[:, 256 * it + 128 * rh : 256 * it + 128 * (rh + 1)],
                                         w1t16[it][:], start=(it == 0), stop=(it == 3))
                dy = [rot.tile([128, D], BF16, tag=f"dy_{rh}", name=f"dy_{rh}") for rh in range(2)]
                for rh in range(2):
                    nc.vector.tensor_tensor(dy[rh][:], p_y[rh][:], vt[rh][:], ALU.subtract)

                dyT = rot.tile([128, 1024], BF16, tag="dyT", name="dyT")
                for ot in range(4):
                    tp = ps2.tile([128, 256], BF16, tag="ptr", name="ptr")
                    for rh in range(2):
                        nc.tensor.transpose(tp[:, 128 * rh : 128 * (rh + 1)],
                                            dy[rh][:, 128 * ot : 128 * (ot + 1)], eye16[:])
                    nc.vector.tensor_copy(dyT[:, 256 * ot : 256 * (ot + 1)], tp[:])

                # P5: g1*c + eta*m1 -> psum; m1 copy; W1 update
                m1n = [st8.tile([128, D], BF16, tag="m1n", name="m1n") for _ in range(4)]
                w1n16n = [st8.tile([128, D], BF16, tag="w1n16n", name="w1n16n") for _ in range(4)]
                for ot in range(4):
                    pg = ps4.tile([128, D], F32, tag="pgrad", name="pgrad")
                    for rh in range(2):
                        nc.tensor.matmul(pg[:], dy[rh][:, 128 * ot : 128 * (ot + 1)], a1c[rh][:],
                                         start=(rh == 0), stop=False)
                    nc.tensor.matmul(pg[:], eyeet[:], m1[ot][:], start=False, stop=True)
                    nc.scalar.activation(m1n[ot][:], pg[:], AF.Copy)
                    nc.vector.scalar_tensor_tensor(
                        w1n32[ot][:], w1n32[ot][:], avc[:, t : t + 1], pg[:], ALU.mult, ALU.add)
                    nc.scalar.activation(w1n16n[ot][:], w1n32[ot][:], AF.Copy)

                # P6: da1 (uses OLD w1n16)
                p_da = [ps2.tile([128, D], F32, tag="pmm", name="pmm") for _ in range(2)]
                for rh in range(2):
                    for ot in range(4):
                        nc.tensor.matmul(p_da[rh][:], dyT[:, 256 * ot + 128 * rh : 256 * ot + 128 * (rh + 1)],
                                         w1n16[ot][:], start=(ot == 0), stop=(ot == 3))
                dh1 = [rot.tile([128, D], BF16, tag=f"dh1_{rh}", name=f"dh1_{rh}") for rh in range(2)]
                for rh in range(2):
                    nc.vector.scalar_tensor_tensor(
                        dh1[rh][:], p_da[rh][:], cvc[:, t : t + 1], ds[rh][:], ALU.mult, ALU.mult)

                # P7: g0T*c + eta*m0 -> psum; m0 copy; W0T update + shadow
                m0n = [st8.tile([128, D], BF16, tag="m0n", name="m0n") for _ in range(4)]
                w0t16n = [st8.tile([128, D], BF16, tag="w0t16n", name="w0t16n") for _ in range(4)]
                for dt in range(4):
                    pg = ps4.tile([128, D], F32, tag="pgrad", name="pgrad")
                    for rh in range(2):
                        nc.tensor.matmul(pg[:], kt[rh][:, 128 * dt : 128 * (dt + 1)], dh1[rh][:],
                                         start=(rh == 0), stop=False)
                    nc.tensor.matmul(pg[:], eyeet[:], m0[dt][:], start=False, stop=True)
                    nc.scalar.activation(m0n[dt][:], pg[:], AF.Copy)
                    nc.vector.scalar_tensor_tensor(
                        w0t32[dt][:], w0t32[dt][:], avc[:, t : t + 1], pg[:], ALU.mult, ALU.add)
                    nc.vector.tensor_copy(w0t16n[dt][:], w0t32[dt][:])

                # W1T for next step
                w1t16n = [st8.tile([128, D], BF16, tag="w1t16n", name="w1t16n") for _ in range(4)]
                for it in range(4):
                    tp = ps2.tile([128, D], BF16, tag="ptr", name="ptr")
                    for ot in range(4):
                        nc.tensor.transpose(tp[:, 128 * ot : 128 * (ot + 1)],
                                            w1n16n[ot][:, 128 * it : 128 * (it + 1)], eye16[:])
                    nc.scalar.activation(w1t16n[it][:], tp[:], AF.Copy)

                m0, m1 = m0n, m1n
                w0t16 = w0t16n
                w1n16, w1t16 = w1n16n, w1t16n

            # ---------- phase C: out = ret @ Wout^T ----------
            woT = [dmaload(rot, [128, D], BF16, woT_d[128 * i : 128 * (i + 1), :], f"woT{i}") for i in range(4)]
            for rt in range(NT):
                rtile = dmaload(rot, [128, D], BF16, ret_d[128 * rt : 128 * (rt + 1), :], "rtile")
                rT = rot.tile([128, D], BF16, tag="retT", name="retT")
                tr128(rT, rtile, None, eye16[:])
                pm = ps2.tile([128, D], F32, tag="pmm", name="pmm")
                for dt in range(4):
                    nc.tensor.matmul(pm[:], rT[:, 128 * dt : 128 * (dt + 1)], woT[dt][:],
                                     start=(dt == 0), stop=(dt == 3))
                ot_ = rot.tile([128, D], F32, tag="otile", name="otile")
                nc.vector.tensor_copy(ot_[:], pm[:])
                nc.sync.dma_start(out_d[128 * rt : 128 * (rt + 1), :], ot_[:])

    nc.compile()
    return nc


def kernel(x, M, mem_W, Wk, Wv, Wq, Wout, Wgd, bgd, Wgl, bgl, Wgm, bgm, gs, gr):
    import ml_dtypes
    from concourse.bass_utils import run_bass_kernel_spmd

    BF = ml_dtypes.bfloat16
    x = np.asarray(x, np.float32)
    M = np.asarray(M, np.float32)
    gs = np.asarray(gs, np.float32)
    gr = np.asarray(gr, np.float32)

    n_steps = int(__import__("os").environ.get("KERNEL_NSTEPS", NSTEP))
    key = n_steps
    if key not in _BUILT:
        _BUILT[key] = _build(n_steps)
    nc = _BUILT[key]

    shared = dict(
        WkT=np.ascontiguousarray(Wk.T).astype(BF),
        WvT=np.ascontiguousarray((Wv * gs[None, :]).T).astype(BF),
        WqT=np.ascontiguousarray((Wq * gr[None, :]).T).astype(BF),
        WoutT=np.ascontiguousarray(Wout.T).astype(BF),
        WgdT=np.ascontiguousarray(Wgd.T / C).astype(BF),
        WglT=np.ascontiguousarray(Wgl.T / C).astype(BF),
        WgmT=np.ascontiguousarray(Wgm.T / C).astype(BF),
        bgd=np.asarray(bgd, np.float32).reshape(D, 1),
        bgl=np.asarray(bgl, np.float32).reshape(D, 1),
        bgm=np.asarray(bgm, np.float32).reshape(D, 1),
        W0T32=np.ascontiguousarray(mem_W[0].T).astype(np.float32),
        W0T16=np.ascontiguousarray(mem_W[0].T).astype(BF),
        W1n32=np.ascontiguousarray(mem_W[1]).astype(np.float32),
        W1n16=np.ascontiguousarray(mem_W[1]).astype(BF),
        W1T16=np.ascontiguousarray(mem_W[1].T).astype(BF),
        EYE16=np.eye(128, dtype=BF),
        EYE32=np.eye(128, dtype=np.float32),
    )
    in_maps = []
    for c in range(NCORE):
        b, par = c // 2, c % 2
        m = dict(shared)
        m["xs"] = np.ascontiguousarray(x[b, par::2, :])
        m["Mb"] = (gs[:, None] * M[b]).astype(BF)
        in_maps.append(m)

    res = run_bass_kernel_spmd(nc, in_maps, list(range(NCORE)))
    out = np.empty((B, S, D), np.float32)
    for c in range(NCORE):
        b, par = c // 2, c % 2
        out[b, par::2, :] = res.results[c]["out"]
    return out

